# revision 1
# baseline (speedup 1.0000x reference)
"""Trainium2 Bass kernel for nn_MultiHeadAttention (B=2, S=2048, D=1024, H=16).

Sharding: 8 cores = 2 batches x 4 head-groups (4 heads each).
Each core receives host-transposed activations x^T (layout [D, S]) for its
batch plus its head-group's slices of the projection weights, computes
  Q^T,K^T = W^T x^T   (per-head [DK, S], heads stacked on partitions)
  V       = x W       (natural [S, DK] per head, +ones column for softmax sums)
  scores^T[kv,q] = K Q^T / sqrt(DK), causal, exp (no max-sub needed: |s|<~4)
  out_av^T = V_aug^T attn^T  (row DK = softmax denominators)
  scale by gate/denominator, project with Wo rows -> partial output [S, D]
Host sums the 4 head-group partials per batch and adds bo.
"""

import os
import numpy as np

P = 128
CHUNK = 512  # q-chunk / matmul moving free dim

_BUILD_CACHE = {}


def _build(S, D, DOUT, HPC, DK, causal, debug=False):
    """Emit the Bass program (same program for all cores; data differs)."""
    import concourse.bass as bass
    import concourse.mybir as mybir
    import concourse.tile as tile
    from concourse import bacc
    from concourse.bass import ds, ts

    fp32 = mybir.dt.float32
    bf16 = mybir.dt.bfloat16
    KC = D // P             # contraction k-chunks for projections
    GCOLS = HPC * DK        # this core's projection output width
    MT = GCOLS // P         # head-pair tiles (2 heads of DK=64 per tile)
    NCH = S // CHUNK        # q-chunks
    TPC = CHUNK // P        # kv tiles per q-chunk (4)
    NKV = S // P            # kv tiles total
    KC2 = GCOLS // P        # out-proj contraction chunks
    NOC = DOUT // CHUNK     # out-proj N chunks
    ST = S // P             # s-tiles
    assert DK * 2 == P and GCOLS % P == 0

    Act = mybir.ActivationFunctionType
    nc = bacc.Bacc()

    xqT = nc.declare_dram_parameter("xqT", [D, S], bf16, isOutput=False)
    xkT = nc.declare_dram_parameter("xkT", [D, S], bf16, isOutput=False)
    xvT = nc.declare_dram_parameter("xvT", [D, S], bf16, isOutput=False)
    wq_d = nc.declare_dram_parameter("wq", [D, GCOLS], bf16, isOutput=False)
    wk_d = nc.declare_dram_parameter("wk", [D, GCOLS], bf16, isOutput=False)
    wv_d = nc.declare_dram_parameter("wv", [D, GCOLS], bf16, isOutput=False)
    wo_d = nc.declare_dram_parameter("wo", [GCOLS, DOUT], bf16, isOutput=False)
    bq_d = nc.declare_dram_parameter("bq", [GCOLS], fp32, isOutput=False)
    bk_d = nc.declare_dram_parameter("bk", [GCOLS], fp32, isOutput=False)
    bv_d = nc.declare_dram_parameter("bv", [1, GCOLS], bf16, isOutput=False)
    wgq_d = nc.declare_dram_parameter("wgq", [D, HPC], fp32, isOutput=False)
    wgk_d = nc.declare_dram_parameter("wgk", [D, HPC], fp32, isOutput=False)
    bg_d = nc.declare_dram_parameter("bg", [1, HPC], fp32, isOutput=False)
    mtri_d = nc.declare_dram_parameter("mtri", [P, P], bf16, isOutput=False)
    outp = nc.declare_dram_parameter("out", [S, DOUT], fp32, isOutput=True)
    if debug:
        qt_d = nc.declare_dram_parameter("qt_dbg", [P, MT, S], fp32, isOutput=True)
        kt_d = nc.declare_dram_parameter("kt_dbg", [P, MT, S], fp32, isOutput=True)
        va_d = nc.declare_dram_parameter("va_dbg", [P, ST, HPC, DK + 1], fp32, isOutput=True)
        g_d = nc.declare_dram_parameter("g_dbg", [P, HPC], fp32, isOutput=True)
        hc_d = nc.declare_dram_parameter("hc_dbg", [P, KC2, S], fp32, isOutput=True)

    scale = 1.0 / float(np.sqrt(DK))

    with tile.TileContext(nc) as tc:
        with (
            tc.tile_pool(name="persist", bufs=1) as pp,
            tc.tile_pool(name="wts", bufs=1) as wp,
        ):
            qt = pp.tile([P, MT, S], bf16, tag="qt")
            kt = pp.tile([P, MT, S], bf16, tag="kt")
            vaug = pp.tile([P, ST, HPC, DK + 1], bf16, tag="vaug")
            hcat = pp.tile([P, KC2, S], bf16, tag="hcat")
            ones = pp.tile([P, P], fp32, tag="ones")
            nc.any.memset(ones[:], 1.0)
            ones_bf = pp.tile([1, P], bf16, tag="ones_bf")
            nc.any.memset(ones_bf[:], 1.0)
            nc.any.memset(vaug[:, :, :, DK : DK + 1], 1.0)

            wq = wp.tile([P, KC, GCOLS], bf16, tag="wq")
            wk = wp.tile([P, KC, GCOLS], bf16, tag="wk")
            wv = wp.tile([P, KC, GCOLS], bf16, tag="wv")
            wo = wp.tile([P, KC2, DOUT], bf16, tag="wo")
            nc.sync.dma_start(wq[:], wq_d.rearrange("(c p) n -> p c n", p=P))
            nc.sync.dma_start(wk[:], wk_d.rearrange("(c p) n -> p c n", p=P))
            nc.sync.dma_start(wv[:], wv_d.rearrange("(c p) n -> p c n", p=P))
            nc.sync.dma_start(wo[:], wo_d.rearrange("(c p) n -> p c n", p=P))
            wgq = wp.tile([P, KC, HPC], fp32, tag="wgq")
            wgk = wp.tile([P, KC, HPC], fp32, tag="wgk")
            nc.sync.dma_start(wgq[:], wgq_d.rearrange("(c p) h -> p c h", p=P))
            nc.sync.dma_start(wgk[:], wgk_d.rearrange("(c p) h -> p c h", p=P))
            bq = wp.tile([P, MT], fp32, tag="bq")
            bk = wp.tile([P, MT], fp32, tag="bk")
            nc.sync.dma_start(bq[:], bq_d.rearrange("(m p) -> p m", p=P))
            nc.sync.dma_start(bk[:], bk_d.rearrange("(m p) -> p m", p=P))
            bv = wp.tile([1, GCOLS], bf16, tag="bv")
            nc.sync.dma_start(bv[:], bv_d[:])
            bg = wp.tile([1, HPC], fp32, tag="bg")
            nc.sync.dma_start(bg[:], bg_d[:])
            mtri = wp.tile([P, P], bf16, tag="mtri")
            nc.sync.dma_start(mtri[:], mtri_d[:])
            gate64 = pp.tile([P, HPC], fp32, tag="gate64")
            lng64 = pp.tile([P, HPC], fp32, tag="lng64")

            # ---------------- Stage A: projections + pooled means + gate
            with (
                tc.tile_pool(name="xsub", bufs=3) as xp,
                tc.tile_pool(name="psa", bufs=3, space="PSUM") as psa,
                tc.tile_pool(name="pmisc", bufs=2) as pm,
            ):
                pooled_nk_q = pm.tile([P, KC, NCH], fp32, tag="pnq")
                pooled_nk_k = pm.tile([P, KC, NCH], fp32, tag="pnk")
                pooled_q = pm.tile([P, KC], fp32, tag="pq")
                pooled_k = pm.tile([P, KC], fp32, tag="pk")

                def proj_T(x_d, w_sb, b_sb, out_sb, pooled_nk):
                    # out_sb[:, m, s] = (W^T x^T)[m-tile] + b  (per-head-pair tiles)
                    x_t = x_d.rearrange("(c p) s -> p c s", p=P)
                    for n in range(NCH):
                        nsl = ds(n * CHUNK, CHUNK)
                        xs = xp.tile([P, KC, CHUNK], bf16, tag="xsub",
                                     name="xsub", bufs=3)
                        nc.sync.dma_start(xs[:], x_t[:, :, nsl])
                        if pooled_nk is not None:
                            for k in range(KC):
                                nc.vector.tensor_reduce(
                                    pooled_nk[:, k, n : n + 1], xs[:, k, :],
                                    mybir.AxisListType.X, mybir.AluOpType.add)
                        for m in range(MT):
                            ps = psa.tile([P, CHUNK], fp32, tag="psa", bufs=3)
                            for k in range(KC):
                                nc.tensor.matmul(
                                    ps[:], w_sb[:, k, ts(m, P)], xs[:, k, :],
                                    start=(k == 0), stop=(k == KC - 1))
                            nc.scalar.activation(
                                out_sb[:, m, nsl], ps[:], Act.Identity,
                                bias=b_sb[:, m : m + 1], scale=1.0)

                proj_T(xqT, wq, bq, qt, pooled_nk_q)
                proj_T(xkT, wk, bk, kt, pooled_nk_k)

                # V natural: lhsT = x_v^T chunks (stationary), rhs = Wv
                xvt_t = xvT.rearrange("(c p) s -> p c s", p=P)
                for n in range(NCH):
                    xs = xp.tile([P, KC, CHUNK], bf16, tag="xsub",
                                 name="xsub", bufs=3)
                    nc.sync.dma_start(xs[:], xvt_t[:, :, ds(n * CHUNK, CHUNK)])
                    for st2 in range(TPC):
                        st = n * TPC + st2
                        ps = psa.tile([P, GCOLS], fp32, tag="psv", bufs=2)
                        for k in range(KC):
                            nc.tensor.matmul(
                                ps[:], xs[:, k, ts(st2, P)], wv[:, k, :],
                                start=(k == 0), stop=False)
                        nc.tensor.matmul(
                            ps[:], ones_bf[0:1, 0:P], bv[:], start=False, stop=True)
                        nc.vector.tensor_copy(
                            vaug[:, st, :, 0:DK],
                            ps.rearrange("p (h d) -> p h d", d=DK))

                # pooled means over S (weights pre-scaled by 1/S on host)
                nc.vector.tensor_reduce(pooled_q[:], pooled_nk_q[:],
                                        mybir.AxisListType.X, mybir.AluOpType.add)
                nc.vector.tensor_reduce(pooled_k[:], pooled_nk_k[:],
                                        mybir.AxisListType.X, mybir.AluOpType.add)

                # gate logits -> sigmoid -> move to partition DK, ln()
                psg = psa.tile([1, HPC], fp32, tag="psg", bufs=1)
                for k in range(KC):
                    nc.tensor.matmul(psg[:], pooled_q[:, k : k + 1], wgq[:, k, :],
                                     start=(k == 0), stop=False)
                for k in range(KC):
                    nc.tensor.matmul(psg[:], pooled_k[:, k : k + 1], wgk[:, k, :],
                                     start=False, stop=False)
                nc.tensor.matmul(psg[:], ones[0:1, 0:1], bg[:],
                                 start=False, stop=True)
                gate0 = pm.tile([1, HPC], fp32, tag="gate0")
                nc.scalar.activation(gate0[:], psg[:], Act.Sigmoid)
                nc.sync.dma_start(gate64[DK : DK + 1, :], gate0[0:1, :])
                nc.scalar.activation(lng64[DK : DK + 1, :], gate64[DK : DK + 1, :],
                                     Act.Ln)

            if debug:
                nc.sync.dma_start(qt_d[:], qt[:])
                nc.sync.dma_start(kt_d[:], kt[:])
                nc.sync.dma_start(va_d[:], vaug[:])
                nc.sync.dma_start(g_d[:], gate64[:])
            # ---------------- Stage B: attention per head-pair
            with (
                tc.tile_pool(name="attn", bufs=4) as ap_,
                tc.tile_pool(name="rows", bufs=3) as rp,
                tc.tile_pool(name="otmp", bufs=3) as op_,
                tc.tile_pool(name="pssc", bufs=4, space="PSUM") as pssc,
                tc.tile_pool(name="psav", bufs=2, space="PSUM") as psav,
                tc.tile_pool(name="psbc", bufs=2, space="PSUM") as psbc,
            ):
                for hp in range(MT):
                    for j in range(NCH):
                        nkv_j = min(TPC * (j + 1), NKV) if causal else NKV
                        pe = psav.tile([DK + 1, CHUNK], fp32, tag="av_e", bufs=1)
                        po = psav.tile([DK + 1, CHUNK], fp32, tag="av_o", bufs=1)
                        for i in range(nkv_j):
                            t = i - TPC * j
                            if causal and t >= 0:
                                Ni = CHUNK - P * t
                                qoff = j * CHUNK + P * t
                            else:
                                Ni = CHUNK
                                qoff = j * CHUNK
                            for half, pav in ((0, pe), (1, po)):
                                hsl = slice(half * DK, (half + 1) * DK)
                                ps = pssc.tile([P, CHUNK], fp32, name="sc",
                                               tag=f"sc{half}", bufs=2)
                                nc.tensor.matmul(
                                    ps[:, :Ni], kt[hsl, hp, ts(i, P)],
                                    qt[hsl, hp, ds(qoff, Ni)],
                                    start=True, stop=True)
                                at = ap_.tile([P, CHUNK], bf16, tag=f"at{half}")
                                nc.scalar.activation(at[:, :Ni], ps[:, :Ni],
                                                     Act.Exp, scale=scale)
                                if causal and t >= 0:
                                    nc.vector.tensor_mul(
                                        at[:, 0:P], at[:, 0:P], mtri[:])
                                nc.tensor.matmul(
                                    pav[:, ds(qoff - j * CHUNK, Ni)],
                                    vaug[:, i, 2 * hp + half, :], at[:, :Ni],
                                    start=(i == 0), stop=(i == nkv_j - 1))
                        # normalize + gate; write headcat^T
                        jsl = ds(j * CHUNK, CHUNK)
                        for half, pav in ((0, pe), (1, po)):
                            h = 2 * hp + half
                            lnr = rp.tile([P, CHUNK], fp32, tag="lnr")
                            rr = rp.tile([P, CHUNK], fp32, tag="rr")
                            nc.scalar.activation(lnr[DK : DK + 1, :],
                                                 pav[DK : DK + 1, :], Act.Ln)
                            nc.scalar.activation(
                                rr[DK : DK + 1, :], lnr[DK : DK + 1, :],
                                Act.Exp, scale=-1.0,
                                bias=lng64[DK : DK + 1, h : h + 1])
                            bcp = psbc.tile([DK, CHUNK], fp32, tag="bc",
                                            bufs=2)
                            nc.tensor.matmul(bcp[:], ones[DK : DK + 1, 0:DK],
                                             rr[DK : DK + 1, :],
                                             start=True, stop=True)
                            bc = rp.tile([DK, CHUNK], fp32, tag="bcs")
                            nc.scalar.copy(bc[:], bcp[:])
                            if half == 0:
                                nc.vector.tensor_mul(hcat[0:DK, hp, jsl],
                                                     pav[0:DK, :], bc[:])
                            else:
                                ot = op_.tile([DK, CHUNK], bf16, tag="ot")
                                nc.vector.tensor_mul(ot[:], pav[0:DK, :], bc[:])
                                nc.sync.dma_start(hcat[DK:P, hp, jsl], ot[:])

            if debug:
                nc.sync.dma_start(hc_d[:], hcat[:])
            # ---------------- Stage C: output projection (partial; host adds bo)
            with (
                tc.tile_pool(name="osb", bufs=3) as ob,
                tc.tile_pool(name="psoc", bufs=3, space="PSUM") as psoc,
            ):
                for st in range(ST):
                    osb = ob.tile([P, DOUT], fp32, tag="osb")
                    for nh in range(NOC):
                        ps = psoc.tile([P, CHUNK], fp32, tag="pso", bufs=3)
                        for k2 in range(KC2):
                            nc.tensor.matmul(
                                ps[:], hcat[:, k2, ts(st, P)],
                                wo[:, k2, ds(nh * CHUNK, CHUNK)],
                                start=(k2 == 0), stop=(k2 == KC2 - 1))
                        nc.vector.tensor_copy(osb[:, ds(nh * CHUNK, CHUNK)], ps[:])
                    nc.sync.dma_start(outp[ts(st, P), :], osb[:])

    nc.compile()
    return nc


def _prep_core_inputs(query, key_, value, Wq, bq, Wk, bk, Wv, bv, Wg, bg, Wo,
                      b, g, S, D, HPC, DK):
    import ml_dtypes
    GCOLS = HPC * DK
    H0 = g * HPC
    cs = slice(H0 * DK, H0 * DK + GCOLS)
    f32 = np.float32
    bf16 = ml_dtypes.bfloat16
    c = np.ascontiguousarray
    return {
        "xqT": c(query[b].T.astype(bf16)),
        "xkT": c(key_[b].T.astype(bf16)),
        "xvT": c(value[b].T.astype(bf16)),
        "wq": c(Wq[:, cs].astype(bf16)),
        "wk": c(Wk[:, cs].astype(bf16)),
        "wv": c(Wv[:, cs].astype(bf16)),
        "wo": c(Wo[cs, :].astype(bf16)),
        "bq": c(bq[cs].astype(f32)),
        "bk": c(bk[cs].astype(f32)),
        "bv": c(bv[cs].astype(bf16)[None, :]),
        "wgq": c((Wg[:D, H0 : H0 + HPC] / S).astype(f32)),
        "wgk": c((Wg[D:, H0 : H0 + HPC] / S).astype(f32)),
        "bg": c(bg[H0 : H0 + HPC].astype(f32)[None, :]),
        "mtri": np.triu(np.ones((P, P), bf16)),
    }


_last_results = None


def kernel(query, key_, value, mask, Wq, bq, Wk, bk, Wv, bv, Wo, bo, Wg, bg):
    global _last_results
    from concourse.bass_utils import run_bass_kernel_spmd

    query = np.asarray(query)
    key_ = np.asarray(key_)
    value = np.asarray(value)
    mask = np.asarray(mask)
    B, S, D = query.shape
    H = np.asarray(bg).shape[0]
    DK = D // H
    DOUT = np.asarray(Wo).shape[1]
    NC_ = 8
    GROUPS = NC_ // B
    HPC = H // GROUPS

    causal = bool(
        np.array_equal(mask[0, 0], np.tril(np.ones((S, S), bool)))
    )
    if not causal:
        assert mask.all(), "only causal or all-true masks supported"

    key = (S, D, DOUT, HPC, DK, causal)
    if key not in _BUILD_CACHE:
        _BUILD_CACHE[key] = _build(*key)
    nc = _BUILD_CACHE[key]

    in_maps = []
    for c in range(NC_):
        b, gidx = divmod(c, GROUPS)
        in_maps.append(_prep_core_inputs(
            query, key_, value, Wq, bq, Wk, bk, Wv, bv, Wg, bg, Wo,
            b, gidx, S, D, HPC, DK))

    res = run_bass_kernel_spmd(nc, in_maps, core_ids=list(range(NC_)))
    _last_results = res

    out = np.zeros((B, S, DOUT), np.float32)
    for c in range(NC_):
        b = c // GROUPS
        out[b] += res.results[c]["out"]
    out += np.asarray(bo).astype(np.float32)
    return out



# revision 12
# speedup vs baseline: 1.2928x; 1.2928x over previous
"""Trainium2 Bass kernel for nn_MultiHeadAttention (B=2, S=2048, D=1024, H=16).

Sharding: 8 cores = 2 batches x 4 head-groups (4 heads each).
Each core receives host-shuffled activations x^T (layout [P, KC, S]) for its
batch plus its head-group's slices of the projection weights, computes
  Q^T,K^T = W^T x^T   (per-head [DK, S], heads stacked on partitions)
  V       = x W       (natural [S, DK] per head, + a ones column per head for
                       the softmax denominators; odd heads store V at columns
                       1..DK so their AV output lands on psum partitions 64..127)
  scores^T[kv,q] = K Q^T / sqrt(DK), causal, exp (no max-sub: |s| < ~4)
  pav^T = V_aug^T attn^T  (denominator rows at psum partitions 64 / 63)
  rr = exp(-ln(Z) + ln(g))  (gate folded via the Exp bias)
  hcat^T = pav^T * broadcast(rr)   (gpsimd partition_broadcast)
  partial out = hcat^T^T @ Wo rows  -> [S, DOUT] fp16
Host sums the 4 head-group partials per batch and adds bo.

vs the original baseline (302 us): one activation-table preload (kills 21
ACT_TABLE_LOADs), denominator/gate normalize without broadcast matmuls or
table thrash, bias-adds on DVE, pooled means on gpsimd/DVE from resident x,
fat contiguous DMA layouts, paired-kv exp (1024-wide), fp16 partial output,
and stage interleaving so the PE stays dense (HAM-warm).
"""

import numpy as np

P = 128
CHUNK = 512  # q-chunk / matmul moving free dim

_BUILD_CACHE = {}


def _build(S, D, DOUT, HPC, DK, causal):
    """Emit the Bass program (same program for all cores; data differs)."""
    import concourse.bass as bass
    import concourse.mybir as mybir
    import concourse.tile as tile
    from concourse import bacc
    from concourse.bass import ds, ts
    from concourse.hw_specs import get_activation_tables

    fp32 = mybir.dt.float32
    fp16 = mybir.dt.float16
    bf16 = mybir.dt.bfloat16
    KC = D // P             # contraction k-chunks for projections
    GCOLS = HPC * DK        # this core's projection output width
    MT = GCOLS // P         # head-pair tiles (2 heads of DK=64 per tile)
    NCH = S // CHUNK        # q-chunks
    TPC = CHUNK // P        # kv tiles per q-chunk (4)
    NKV = S // P            # kv tiles total
    KC2 = GCOLS // P        # out-proj contraction chunks
    NOC = DOUT // CHUNK     # out-proj N chunks
    ST = S // P             # s-tiles
    HP2 = HPC // 2
    assert DK * 2 == P and GCOLS % P == 0

    Act = mybir.ActivationFunctionType
    nc = bacc.Bacc()

    xq_d = nc.declare_dram_parameter("xq", [P, KC, S], bf16, isOutput=False)
    xk_d = nc.declare_dram_parameter("xk", [P, KC, S], bf16, isOutput=False)
    xv_d = nc.declare_dram_parameter("xv", [P, KC, S], bf16, isOutput=False)
    wq_d = nc.declare_dram_parameter("wq", [P, KC, GCOLS], bf16, isOutput=False)
    wk_d = nc.declare_dram_parameter("wk", [P, KC, GCOLS], bf16, isOutput=False)
    wv_d = nc.declare_dram_parameter("wv", [P, KC, GCOLS], bf16, isOutput=False)
    wo_d = nc.declare_dram_parameter("wo", [P, KC2, DOUT], bf16, isOutput=False)
    bq_d = nc.declare_dram_parameter("bq", [P, MT], fp32, isOutput=False)
    bk_d = nc.declare_dram_parameter("bk", [P, MT], fp32, isOutput=False)
    bv_d = nc.declare_dram_parameter("bv", [1, GCOLS], bf16, isOutput=False)
    wgq_d = nc.declare_dram_parameter("wgq", [P, KC, HPC], fp32, isOutput=False)
    wgk_d = nc.declare_dram_parameter("wgk", [P, KC, HPC], fp32, isOutput=False)
    bg_d = nc.declare_dram_parameter("bg", [1, HPC], fp32, isOutput=False)
    mtri_d = nc.declare_dram_parameter("mtri", [P, P], bf16, isOutput=False)
    outp = nc.declare_dram_parameter("out", [S, DOUT], fp16, isOutput=True)

    scale = 1.0 / float(np.sqrt(DK))

    # natural_log_exp_and_others: covers Exp, Ln, Identity, Copy -- the only
    # scalar-engine functions this kernel uses.  Preloading it once stops the
    # compiler's per-function table churn (exp_and_others <-> natural_log).
    table_names = list(get_activation_tables(nc.m.arch))
    nle_set_id = table_names.index("natural_log_exp_and_others")

    with tile.TileContext(nc) as tc:
        nc.scalar.add_instruction(
            mybir.InstLoadActFuncSet(
                name=nc.get_next_instruction_name(),
                act_func_set_id=nle_set_id, ins=[], outs=[]))

        with (
            tc.tile_pool(name="persist", bufs=1) as pp,
            tc.tile_pool(name="work", bufs=1) as wkp,
            tc.tile_pool(name="ps", bufs=1, space="PSUM") as psp,
        ):
            # ------------- persistent tiles + weight/bias loads
            wq = pp.tile([P, KC, GCOLS], bf16, tag="wq")
            wk = pp.tile([P, KC, GCOLS], bf16, tag="wk")
            wv = pp.tile([P, KC, GCOLS], bf16, tag="wv")
            wo = pp.tile([P, KC2, DOUT], bf16, tag="wo")
            nc.sync.dma_start(wq[:], wq_d[:])
            nc.sync.dma_start(wk[:], wk_d[:])
            nc.sync.dma_start(wv[:], wv_d[:])
            nc.sync.dma_start(wo[:], wo_d[:])
            wgq = pp.tile([P, KC, HPC], fp32, tag="wgq")
            wgk = pp.tile([P, KC, HPC], fp32, tag="wgk")
            nc.sync.dma_start(wgq[:], wgq_d[:])
            nc.sync.dma_start(wgk[:], wgk_d[:])
            bq = pp.tile([P, MT], fp32, tag="bq")
            bk = pp.tile([P, MT], fp32, tag="bk")
            nc.sync.dma_start(bq[:], bq_d[:])
            nc.sync.dma_start(bk[:], bk_d[:])
            bv = pp.tile([1, GCOLS], bf16, tag="bv")
            nc.sync.dma_start(bv[:], bv_d[:])
            bg = pp.tile([1, HPC], fp32, tag="bg")
            nc.sync.dma_start(bg[:], bg_d[:])
            mtri = pp.tile([P, P], bf16, tag="mtri")
            nc.sync.dma_start(mtri[:], mtri_d[:])

            xq_r = pp.tile([P, KC, S], bf16, tag="xq_r")
            xk_r = pp.tile([P, KC, S], bf16, tag="xk_r")
            xv_r = pp.tile([P, KC, S], bf16, tag="xv_r")
            # per-c slabs so pooling can start while later slabs stream in
            for c in range(0, KC, 2):
                nc.sync.dma_start(xk_r[:, c : c + 2, :], xk_d[:, c : c + 2, :])
            for c in range(0, KC, 2):
                nc.sync.dma_start(xq_r[:, c : c + 2, :], xq_d[:, c : c + 2, :])
            for c in range(0, KC, 4):
                nc.sync.dma_start(xv_r[:, c : c + 4, :], xv_d[:, c : c + 4, :])

            qt = pp.tile([P, MT, S], bf16, tag="qt")
            kt = pp.tile([P, MT, S], bf16, tag="kt")
            # vaug[:, st, h, :]: V at cols 0..DK-1, ones col at DK so the AV
            # matmul's output row DK accumulates the softmax denominators.
            vaug = pp.tile([P, ST, HPC, DK + 1], bf16, tag="vaug")
            hcat = pp.tile([P, KC2, S], bf16, tag="hcat")
            ones1 = pp.tile([1, P], bf16, tag="ones1")
            onesg = pp.tile([1, 1], fp32, tag="onesg")
            nc.any.memset(ones1[:], 1.0)
            nc.any.memset(onesg[:], 1.0)
            nc.any.memset(vaug[:, :, :, DK : DK + 1], 1.0)

            pooled_q = pp.tile([P, KC], fp32, tag="pooled_q")
            pooled_k = pp.tile([P, KC], fp32, tag="pooled_k")

            # ------------- pooled column sums (gate), from resident x
            # (wgq/wgk are pre-scaled by 1/S on the host).  xq sums ride the
            # idle-early scalar engine via Copy+accum_out; xk on DVE.
            pscr = wkp.tile([P, S], bf16, tag="pscr")
            for c in range(KC):
                nc.scalar.activation(
                    pscr[:, :], xq_r[:, c, :], Act.Copy,
                    accum_out=pooled_q[:, c : c + 1])
                nc.vector.tensor_reduce(
                    pooled_k[:, c : c + 1], xk_r[:, c, :],
                    mybir.AxisListType.X, mybir.AluOpType.add)

            def proj_chunk(n, x_r, w_sb, b_sb, out_sb):
                nsl = ds(n * CHUNK, CHUNK)
                ps = psp.tile([P, 2, CHUNK], fp32, tag="sc", bufs=2, name="ps")
                for m in range(MT):
                    for k in range(KC):
                        nc.tensor.matmul(
                            ps[:, m, :], w_sb[:, k, ts(m, P)], x_r[:, k, nsl],
                            start=(k == 0), stop=(k == KC - 1))
                for m in range(MT):
                    nc.vector.tensor_scalar(
                        out_sb[:, m, nsl], ps[:, m, :], b_sb[:, m : m + 1],
                        None, mybir.AluOpType.add)

            # K projection (all chunks), then Q chunk 0
            for n in range(NCH):
                proj_chunk(n, xk_r, wk, bk, kt)
            proj_chunk(0, xq_r, wq, bq, qt)

            # V projection -> vaug
            for st in range(ST):
                pv = psp.tile([P, GCOLS], fp32, tag="pe" if st % 2 == 0 else "po",
                              bufs=2, name="pv")
                for k in range(KC):
                    nc.tensor.matmul(
                        pv[:], xv_r[:, k, ts(st, P)], wv[:, k, :],
                        start=(k == 0), stop=False)
                nc.tensor.matmul(pv[:], ones1[0:1, 0:P], bv[:],
                                 start=False, stop=True)
                nc.vector.tensor_copy(
                    vaug[:, st, :, 0:DK],
                    pv.rearrange("p (h d) -> p h d", d=DK))

            # ------------- gate: sigmoid via exp/ln (stays in one table set)
            # lng = logits - ln(1 + exp(logits)) = ln(sigmoid(logits))
            psg = psp.tile([1, HPC], fp32, tag="pe", bufs=2, name="psg")
            for c in range(KC):
                nc.tensor.matmul(psg[:], pooled_q[:, c : c + 1], wgq[:, c, :],
                                 start=(c == 0), stop=False)
            for c in range(KC):
                nc.tensor.matmul(psg[:], pooled_k[:, c : c + 1], wgk[:, c, :],
                                 start=False, stop=False)
            nc.tensor.matmul(psg[:], onesg[0:1, 0:1], bg[:],
                             start=False, stop=True)
            ge = wkp.tile([1, HPC], fp32, tag="ge")
            gu = wkp.tile([1, HPC], fp32, tag="gu")
            gv = wkp.tile([1, HPC], fp32, tag="gv")
            lng0 = wkp.tile([1, HPC], fp32, tag="lng0")
            nc.scalar.activation(ge[:], psg[:], Act.Exp)
            nc.vector.tensor_scalar(gu[:], ge[:], 1.0, None,
                                    mybir.AluOpType.add)
            nc.scalar.activation(gv[:], gu[:], Act.Ln)
            nc.vector.tensor_tensor(lng0[:], psg[:], gv[:],
                                    mybir.AluOpType.subtract)

            # ------------- attention + out-projection, chunk by chunk
            # scores run in 64x128 PE-tiling mode (contraction 64, halves on
            # row-tiles 0/64); AV runs in 128x128 mode.  Batch several pairs
            # of each so the PE reconfigures (drains) once per batch instead
            # of on every matmul.
            for j in range(NCH):
                jsl = ds(j * CHUNK, CHUNK)
                nkv_j = min(TPC * (j + 1), NKV) if causal else NKV
                npairs = (nkv_j + 1) // 2
                for hp in range(MT):
                    pe_t = psp.tile([P, CHUNK], fp32, tag="pe", bufs=2,
                                    name="pe_t")
                    po_t = psp.tile([P, CHUNK], fp32, tag="po", bufs=2,
                                    name="po_t")
                    pav = (pe_t, po_t)
                    for bstart in range(0, npairs, 2):
                        bpairs = min(2, npairs - bstart)
                        batch = []  # (half, at_t, subs)
                        # scores + exp phase (64-contraction mode)
                        for ipo in range(bpairs):
                            ip = bstart + ipo
                            sl_n = min(2, nkv_j - 2 * ip)
                            subs = []
                            for sl in range(sl_n):
                                i = 2 * ip + sl
                                t = i - TPC * j
                                if causal and t >= 0:
                                    subs.append((i, t, CHUNK - P * t, P * t))
                                else:
                                    subs.append((i, t, CHUNK, 0))
                            sc_p = [
                                psp.tile([P, 2, CHUNK], fp32, tag="sc",
                                         bufs=2, name="sc_t")
                                for _ in range(2)]
                            at_p = [
                                wkp.tile([P, 2, CHUNK], bf16, tag="at",
                                         bufs=8, name="at_t")
                                for _ in range(2)]
                            for sl, (i, t, Ni, qoff) in enumerate(subs):
                                for half in range(2):
                                    hsl = slice(half * DK, (half + 1) * DK)
                                    nc.tensor.matmul(
                                        sc_p[half][:, sl, :Ni],
                                        kt[hsl, hp, ts(i, P)],
                                        qt[hsl, hp, ds(j * CHUNK + qoff, Ni)],
                                        start=True, stop=True)
                            full = all(Ni == CHUNK for (i, t, Ni, qoff) in subs)
                            for half in range(2):
                                if full:
                                    nc.scalar.activation(
                                        at_p[half][:, 0:sl_n, :],
                                        sc_p[half][:, 0:sl_n, :],
                                        Act.Exp, scale=scale)
                                else:
                                    for sl, (i, t, Ni, qoff) in enumerate(subs):
                                        nc.scalar.activation(
                                            at_p[half][:, sl, :Ni],
                                            sc_p[half][:, sl, :Ni],
                                            Act.Exp, scale=scale)
                                for sl, (i, t, Ni, qoff) in enumerate(subs):
                                    if causal and t >= 0:
                                        nc.vector.tensor_mul(
                                            at_p[half][:, sl, 0:P],
                                            at_p[half][:, sl, 0:P], mtri[:])
                                batch.append((half, at_p[half], subs))
                        # AV phase (128-contraction mode)
                        for half, at_t, subs in batch:
                            for sl, (i, t, Ni, qoff) in enumerate(subs):
                                nc.tensor.matmul(
                                    pav[half][0 : DK + 1, ds(qoff, Ni)],
                                    vaug[:, i, 2 * hp + half, :],
                                    at_t[:, sl, :Ni],
                                    start=(i == 0), stop=(i == nkv_j - 1))

                    # normalize + gate both halves of this head-pair.
                    # rr = exp(-ln Z + ln g) lands on partition 0 (DVE/ACT
                    # support shifted partition bases; partition_broadcast
                    # only works from base 0 on HW).
                    for half in range(2):
                        h = 2 * hp + half
                        lnz = wkp.tile([1, CHUNK], fp32, tag="lnz", bufs=2,
                                       name="lnz")
                        rr = wkp.tile([1, CHUNK], fp32, tag="rr", bufs=2,
                                      name="rr")
                        bc = wkp.tile([P, CHUNK], fp32, tag="bc", bufs=2,
                                      name="bc")
                        nc.scalar.activation(
                            lnz[0:1, :], pav[half][DK : DK + 1, :], Act.Ln)
                        nc.scalar.activation(
                            rr[0:1, :], lnz[0:1, :],
                            Act.Exp, scale=-1.0,
                            bias=lng0[0:1, h : h + 1])
                        nc.gpsimd.partition_broadcast(bc[0:DK, :], rr[0:1, :])
                        if half == 0:
                            nc.vector.tensor_mul(
                                hcat[0:DK, hp, jsl],
                                pav[half][0:DK, :], bc[0:DK, :])
                        else:
                            nc.vector.tensor_mul(
                                hcat[DK:P, hp, jsl],
                                pav[half][0:DK, :], bc[0:DK, :])

                # prefetch next q-chunk's Q projection
                if j + 1 < NCH:
                    proj_chunk(j + 1, xq_r, wq, bq, qt)

                # out-projection for this q-chunk (host adds bo)
                for st in range(TPC * j, TPC * (j + 1)):
                    osb = wkp.tile([P, DOUT], fp16, tag="osb", bufs=3,
                                   name="osb")
                    for nh in range(NOC):
                        pc = psp.tile([P, CHUNK], fp32,
                                      tag="pe" if nh == 0 else "po", bufs=2,
                                      name="pc")
                        for k2 in range(KC2):
                            nc.tensor.matmul(
                                pc[:], hcat[:, k2, ts(st, P)],
                                wo[:, k2, ds(nh * CHUNK, CHUNK)],
                                start=(k2 == 0), stop=(k2 == KC2 - 1))
                        nc.vector.tensor_copy(osb[:, ds(nh * CHUNK, CHUNK)],
                                              pc[:])
                    nc.sync.dma_start(outp[ts(st, P), :], osb[:])

    nc.compile()
    return nc


def _prep_core_inputs(query, key_, value, Wq, bq, Wk, bk, Wv, bv, Wg, bg, Wo,
                      b, g, S, D, HPC, DK):
    import ml_dtypes
    GCOLS = HPC * DK
    KC = D // P
    KC2 = GCOLS // P
    MT = GCOLS // P
    H0 = g * HPC
    cs = slice(H0 * DK, H0 * DK + GCOLS)
    f32 = np.float32
    bf16 = ml_dtypes.bfloat16
    c = np.ascontiguousarray

    def shuf_rows(a, nchunks):
        # [nchunks*P, N] -> [P, nchunks, N] with row r = chunk*P + p
        return c(a.reshape(nchunks, P, -1).transpose(1, 0, 2))

    return {
        "xq": shuf_rows(query[b].T.astype(bf16), KC),
        "xk": shuf_rows(key_[b].T.astype(bf16), KC),
        "xv": shuf_rows(value[b].T.astype(bf16), KC),
        "wq": shuf_rows(Wq[:, cs].astype(bf16), KC),
        "wk": shuf_rows(Wk[:, cs].astype(bf16), KC),
        "wv": shuf_rows(Wv[:, cs].astype(bf16), KC),
        "wo": shuf_rows(Wo[cs, :].astype(bf16), KC2),
        "bq": c(bq[cs].astype(f32).reshape(MT, P).T),
        "bk": c(bk[cs].astype(f32).reshape(MT, P).T),
        "bv": c(bv[cs].astype(bf16)[None, :]),
        "wgq": shuf_rows((Wg[:D, H0 : H0 + HPC] / S).astype(f32), KC),
        "wgk": shuf_rows((Wg[D:, H0 : H0 + HPC] / S).astype(f32), KC),
        "bg": c(bg[H0 : H0 + HPC].astype(f32)[None, :]),
        "mtri": np.triu(np.ones((P, P), bf16)),
    }


_last_results = None


def kernel(query, key_, value, mask, Wq, bq, Wk, bk, Wv, bv, Wo, bo, Wg, bg):
    global _last_results
    from concourse.bass_utils import run_bass_kernel_spmd

    query = np.asarray(query)
    key_ = np.asarray(key_)
    value = np.asarray(value)
    mask = np.asarray(mask)
    B, S, D = query.shape
    H = np.asarray(bg).shape[0]
    DK = D // H
    DOUT = np.asarray(Wo).shape[1]
    NC_ = 8
    GROUPS = NC_ // B
    HPC = H // GROUPS

    causal = bool(
        np.array_equal(mask[0, 0], np.tril(np.ones((S, S), bool)))
    )
    if not causal:
        assert mask.all(), "only causal or all-true masks supported"

    key = (S, D, DOUT, HPC, DK, causal)
    if key not in _BUILD_CACHE:
        _BUILD_CACHE[key] = _build(*key)
    nc = _BUILD_CACHE[key]

    in_maps = []
    for c in range(NC_):
        b, gidx = divmod(c, GROUPS)
        in_maps.append(_prep_core_inputs(
            query, key_, value, Wq, bq, Wk, bk, Wv, bv, Wg, bg, Wo,
            b, gidx, S, D, HPC, DK))

    res = run_bass_kernel_spmd(nc, in_maps, core_ids=list(range(NC_)))
    _last_results = res

    out = np.zeros((B, S, DOUT), np.float32)
    for c in range(NC_):
        b = c // GROUPS
        out[b] += res.results[c]["out"].astype(np.float32)
    out += np.asarray(bo).astype(np.float32)
    return out
